# revision 12
# baseline (speedup 1.0000x reference)
"""GCN embedding kernel for 8 Trainium2 NeuronCores.

Strategy (edge-cut node partitioning, pull model):
  - Nodes are sharded contiguously across 8 cores (6250 each). Each core owns
    the edges whose *destination* lies in its shard.
  - Per core, edges are grouped by 128-node dst tile (49 tiles). Because
    dma_gather indices are int16, each tile's edges are split into "low"
    (src < 32768) and "high" (src >= 32768) segments; each segment is padded
    to a uniform ML / MH chunks of 128 edges so every core runs the identical
    SPMD program.
  - Tables are stored bf16 with 128 columns (64 real + 64 pad) so each row is
    the 256B minimum dma_gather element. Gathers are batched over groups of 7
    dst tiles (one call per table half per group) round-robined over 4 SWDGE
    queues; gathered edge i lands at partition i%128, free block i//128. Each
    gather buffer is scaled by the per-edge weight in one broadcast
    tensor_tensor op.
  - Aggregation: one-hot bf16 selection matrices S[e, j] = (dstlocal_e == j)
    are built four 128-edge chunks at a time with a single broadcast is_equal
    against an iota tile; the scatter-add becomes bf16 PE matmuls
    S^T @ scaled_rows accumulated in fp32 PSUM.
  - Tables are pre-scaled by dinv so the per-edge norm reduces to w_e, with
    dinv[dst] applied on the aggregated output tile (symmetric-norm
    factorization: out[d] = dinv[d] * sum_e w_e * dinv[src_e] * h[src_e]).
  - conv2 aggregates the 64-dim h1 first and applies W2 afterwards
    (segment-sum is linear), halving conv2 gather traffic.
  - table1 = dinv*(x@W1) is computed fully on every core (replicated) from a
    bf16 copy of x using DMA-transpose loads - cheaper than an AllGather.
  - Halo exchange: a single AllGather of the conv1 output shard with the
    per-shard BN statistics appended as two extra rows (no separate
    AllReduce); each core then computes BN+relu+dinv for the full graph
    locally (7 tiles per vector op) to form the conv2 table.
"""

import os

import numpy as np

N = 50000
IN_CH = 128
HID = 64
OUT_CH = 128
NCORES = 8
SHARD = N // NCORES  # 6250
P = 128
TPC = (SHARD + P - 1) // P  # 49 tiles per core
NLAST = SHARD - (TPC - 1) * P  # 106 rows in last tile
BN_EPS = 1e-5
HALF = 32768  # int16 index limit for dma_gather
GS = 7  # dst tiles per gather group
NG = TPC // GS  # 7 groups
SH2 = SHARD + 2  # shard rows + [sum; sumsq] stat rows
SB = 4  # chunks per S-build op
NPAD = 50048  # N padded to tile multiple
GT = NPAD // P  # 391 plain global tiles
TW = 128  # table row width (bf16): 64 data + 64 pad = 256B

LAST_RESULTS = None

# ablation knobs for benchmarking (affect program build)
ABL_PHASEB = True
ABL_CONV1 = True
ABL_C2 = True
ABL_CONV2 = True
ABL_COLL = True

_PROGRAM_CACHE = {}


def _wrap16(flat):
    """dma_gather index layout: linear i -> (partition i%16, col i//16),
    replicated to all 128 partitions (8 Q7 cores x 16 partitions)."""
    arr = flat.reshape(-1, 16).T  # [16, n/16]
    return np.ascontiguousarray(np.tile(arr, (8, 1)))  # [128, n/16]


def _host_prep(edge_src, edge_dst, edge_weight):
    """Partition/sort/pad the edge list. Returns per-core arrays and ML/MH."""
    src = np.concatenate([edge_src.astype(np.int64), np.arange(N, dtype=np.int64)])
    dst = np.concatenate([edge_dst.astype(np.int64), np.arange(N, dtype=np.int64)])
    w = np.concatenate(
        [edge_weight.astype(np.float32), np.ones(N, dtype=np.float32)]
    )

    deg = np.bincount(dst, weights=w.astype(np.float64), minlength=N).astype(
        np.float32
    )

    core = dst // SHARD
    ld = dst - core * SHARD
    t = ld // P
    dl = (ld - t * P).astype(np.float32)
    hi = (src >= HALF).astype(np.int64)

    order = np.lexsort((src, hi, t, core))
    src_s, w_s, dl_s = src[order], w[order], dl[order]
    key2 = ((core[order] * TPC + t[order]) * 2 + hi[order])

    counts = np.bincount(key2, minlength=NCORES * TPC * 2)
    ML = int((counts[0::2].max() + P - 1) // P)
    MH = int((counts[1::2].max() + P - 1) // P)
    M2 = ML + MH

    CM = NCORES * TPC
    seg_starts = np.zeros(NCORES * TPC * 2, dtype=np.int64)
    seg_starts[1:] = np.cumsum(counts)[:-1]
    within = np.arange(len(key2), dtype=np.int64) - seg_starts[key2]
    ct = key2 // 2
    pos = ct * (M2 * P) + np.where(key2 % 2 == 0, 0, ML * P) + within

    SRC = np.zeros(CM * M2 * P, dtype=np.int64)
    DL = np.zeros(CM * M2 * P, dtype=np.float32)
    WV = np.zeros(CM * M2 * P, dtype=np.float32)
    SRC[pos] = src_s
    DL[pos] = dl_s
    WV[pos] = w_s

    SRC = SRC.reshape(NCORES, TPC, M2, P)
    DL = DL.reshape(NCORES, TPC, M2, P)
    WV = WV.reshape(NCORES, TPC, M2, P)

    SRC[:, :, ML:, :] = np.maximum(SRC[:, :, ML:, :] - HALF, 0)

    idxlo_h, idxhi_h, dl_h, wvlo_h, wvhi_h = [], [], [], [], []
    for c in range(NCORES):
        lo = SRC[c, :, :ML, :].reshape(NG, GS * ML * P)
        hi_ = SRC[c, :, ML:, :].reshape(NG, GS * MH * P)
        idxlo_h.append(
            np.concatenate([_wrap16(lo[g].astype(np.int16)) for g in range(NG)],
                           axis=1))
        idxhi_h.append(
            np.concatenate([_wrap16(hi_[g].astype(np.int16)) for g in range(NG)],
                           axis=1))
        dl_h.append(np.ascontiguousarray(DL[c].reshape(TPC * M2, P).T))
        import ml_dtypes
        wvlo_h.append(np.ascontiguousarray(
            WV[c, :, :ML, :].reshape(TPC * ML, P).T).astype(ml_dtypes.bfloat16))
        wvhi_h.append(np.ascontiguousarray(
            WV[c, :, ML:, :].reshape(TPC * MH, P).T).astype(ml_dtypes.bfloat16))

    deg_h = []
    for c in range(NCORES):
        d = deg[c * SHARD:(c + 1) * SHARD]
        dp = np.ones(TPC * P, dtype=np.float32)
        dp[:SHARD] = d
        deg_h.append(np.ascontiguousarray(dp.reshape(TPC, P).T))

    # full-graph deg in core-block tile layout [128, c*TPC+t] (for phase C2)
    dall = np.ones(NCORES * TPC * P, dtype=np.float32)
    for c in range(NCORES):
        dall[c * TPC * P: c * TPC * P + SHARD] = deg[c * SHARD:(c + 1) * SHARD]
    degall_cb = np.ascontiguousarray(dall.reshape(NCORES * TPC, P).T)

    # full-graph deg in plain global tile layout [128, GT] (for phase B)
    dpl = np.ones(NPAD, dtype=np.float32)
    dpl[:N] = deg
    degall_pl = np.ascontiguousarray(dpl.reshape(GT, P).T)

    return (idxlo_h, idxhi_h, dl_h, wvlo_h, wvhi_h, deg_h, degall_cb,
            degall_pl, ML, MH)


def _build_program(ML, MH):
    import concourse.bacc as bacc
    import concourse.mybir as mybir
    import concourse.tile as tile
    from concourse.masks import make_identity

    f32 = mybir.dt.float32
    bf16 = mybir.dt.bfloat16
    i16 = mybir.dt.int16
    i32 = mybir.dt.int32
    Alu = mybir.AluOpType
    Act = mybir.ActivationFunctionType

    M2 = ML + MH
    CLO = GS * ML * P // 16
    CHI = GS * MH * P // 16
    CT = NCORES * TPC  # 392 core-block tiles

    nc = bacc.Bacc("TRN2", target_bir_lowering=False, debug=False,
                   num_devices=NCORES, num_swdge_queues=4)

    x_in = nc.dram_tensor("x_in", [NPAD, IN_CH], bf16, kind="ExternalInput")
    idxlo_in = nc.dram_tensor("idxlo_in", [P, NG * CLO], i16,
                              kind="ExternalInput")
    idxhi_in = nc.dram_tensor("idxhi_in", [P, NG * CHI], i16,
                              kind="ExternalInput")
    dl_in = nc.dram_tensor("dl_in", [P, TPC * M2], f32, kind="ExternalInput")
    wvlo_in = nc.dram_tensor("wvlo_in", [P, TPC * ML], bf16,
                             kind="ExternalInput")
    wvhi_in = nc.dram_tensor("wvhi_in", [P, TPC * MH], bf16,
                             kind="ExternalInput")
    deg_in = nc.dram_tensor("deg_in", [P, TPC], f32, kind="ExternalInput")
    degcb_in = nc.dram_tensor("degcb_in", [P, CT], f32, kind="ExternalInput")
    degpl_in = nc.dram_tensor("degpl_in", [P, GT], f32, kind="ExternalInput")
    mask_in = nc.dram_tensor("mask_in", [P, 2], f32, kind="ExternalInput")
    w1_in = nc.dram_tensor("w1_in", [IN_CH, HID], bf16, kind="ExternalInput")
    w2_in = nc.dram_tensor("w2_in", [HID, OUT_CH], f32, kind="ExternalInput")
    b1_in = nc.dram_tensor("b1_in", [HID, 1], f32, kind="ExternalInput")
    b2_in = nc.dram_tensor("b2_in", [OUT_CH, 1], f32, kind="ExternalInput")
    g_in = nc.dram_tensor("g_in", [HID, 1], f32, kind="ExternalInput")
    be_in = nc.dram_tensor("be_in", [HID, 1], f32, kind="ExternalInput")

    y_out = nc.dram_tensor("y_out", [TPC * P, OUT_CH], f32,
                           kind="ExternalOutput")

    rg = [list(range(NCORES))]
    NB = (M2 + SB - 1) // SB

    with tile.TileContext(nc) as tc:
        with (
            tc.tile_pool(name="const", bufs=1) as cpool,
            tc.tile_pool(name="dram", bufs=1, space="DRAM") as dpool,
            tc.tile_pool(name="xb", bufs=4) as xpool,
            tc.tile_pool(name="gb", bufs=2) as gpool,
            tc.tile_pool(name="sb", bufs=8) as spool,
            tc.tile_pool(name="ob", bufs=6) as opool,
            tc.tile_pool(name="psA", bufs=4, space="PSUM") as psA,
            tc.tile_pool(name="psB", bufs=2, space="PSUM") as psB,
            tc.tile_pool(name="psS", bufs=1, space="PSUM") as psS,
        ):
            # ---- constants / persistent state ----
            w1s = cpool.tile([IN_CH, HID], bf16)
            nc.sync.dma_start(out=w1s[:], in_=w1_in[:])
            w2s = cpool.tile([HID, OUT_CH], f32)
            nc.sync.dma_start(out=w2s[:], in_=w2_in[:])
            b1c = cpool.tile([HID, 1], f32)
            nc.sync.dma_start(out=b1c[:], in_=b1_in[:])
            b2c = cpool.tile([OUT_CH, 1], f32)
            nc.sync.dma_start(out=b2c[:], in_=b2_in[:])
            gc = cpool.tile([HID, 1], f32)
            nc.sync.dma_start(out=gc[:], in_=g_in[:])
            bec = cpool.tile([HID, 1], f32)
            nc.sync.dma_start(out=bec[:], in_=be_in[:])
            idxlo = cpool.tile([P, NG * CLO], i16)
            nc.sync.dma_start(out=idxlo[:], in_=idxlo_in[:])
            idxhi = cpool.tile([P, NG * CHI], i16)
            nc.sync.dma_start(out=idxhi[:], in_=idxhi_in[:])
            dls = cpool.tile([P, TPC * M2], f32)
            nc.sync.dma_start(out=dls[:], in_=dl_in[:])
            wvlos = cpool.tile([P, TPC * ML], bf16)
            nc.sync.dma_start(out=wvlos[:], in_=wvlo_in[:])
            wvhis = cpool.tile([P, TPC * MH], bf16)
            nc.sync.dma_start(out=wvhis[:], in_=wvhi_in[:])
            masks = cpool.tile([P, 2], f32)
            nc.sync.dma_start(out=masks[:], in_=mask_in[:])

            degs = cpool.tile([P, TPC], f32)
            nc.sync.dma_start(out=degs[:], in_=deg_in[:])
            dinvs = cpool.tile([P, TPC], f32)
            nc.scalar.activation(out=dinvs[:], in_=degs[:], func=Act.Sqrt)
            nc.vector.reciprocal(out=dinvs[:], in_=dinvs[:])
            degcbs = cpool.tile([P, CT], f32)
            nc.sync.dma_start(out=degcbs[:], in_=degcb_in[:])
            dinvcb = cpool.tile([P, CT], f32)
            nc.scalar.activation(out=dinvcb[:], in_=degcbs[:], func=Act.Sqrt)
            nc.vector.reciprocal(out=dinvcb[:], in_=dinvcb[:])
            degpls = cpool.tile([P, GT], f32)
            nc.sync.dma_start(out=degpls[:], in_=degpl_in[:])
            dinvpl = cpool.tile([P, GT], f32)
            nc.scalar.activation(out=dinvpl[:], in_=degpls[:], func=Act.Sqrt)
            nc.vector.reciprocal(out=dinvpl[:], in_=dinvpl[:])

            ident = cpool.tile([P, P], f32)
            make_identity(nc, ident[:])
            iota_i = cpool.tile([P, SB * P], i32)
            nc.gpsimd.iota(iota_i[:], [[0, SB], [1, P]], channel_multiplier=0)
            iotaf = cpool.tile([P, SB * P], f32)
            nc.vector.tensor_copy(out=iotaf[:], in_=iota_i[:])

            b1bc_p = psB.tile([P, HID], f32, tag="ps")
            nc.tensor.transpose(b1bc_p[:], b1c[:].to_broadcast([HID, P]),
                                ident[:HID, :HID])
            b1bc = cpool.tile([P, HID], f32)
            nc.vector.tensor_copy(out=b1bc[:], in_=b1bc_p[:])
            b2bc_p = psB.tile([P, OUT_CH], f32, tag="ps")
            nc.tensor.transpose(b2bc_p[:], b2c[:].to_broadcast([OUT_CH, P]),
                                ident[:])
            b2bc = cpool.tile([P, OUT_CH], f32)
            nc.vector.tensor_copy(out=b2bc[:], in_=b2bc_p[:])

            t1full = dpool.tile([NPAD, TW], bf16)
            o1sh = dpool.tile([SH2, HID], f32)
            o1full = dpool.tile([NCORES * SH2 + 32, HID], f32)
            t2full = dpool.tile([NPAD, TW], bf16)

            # ---- phase B (replicated): t1full = dinv * (x @ W1), bf16 ----
            for g in range(GT if ABL_PHASEB else 8):
                xT = xpool.tile([P, P], bf16, tag="xT")
                nc.sync.dma_start(out=xT[:], in_=x_in[g * P:(g + 1) * P, :],
                                  transpose=True)
                h_p = psB.tile([P, HID], f32, tag="ps")
                nc.tensor.matmul(h_p[:], lhsT=xT[:], rhs=w1s[:],
                                 start=True, stop=True)
                t1t = opool.tile([P, HID], bf16, tag="t1t")
                nc.vector.tensor_scalar(
                    out=t1t[:], in0=h_p[:], scalar1=dinvpl[:, g:g + 1],
                    scalar2=None, op0=Alu.mult)
                nc.sync.dma_start(out=t1full[g * P:(g + 1) * P, 0:HID],
                                  in_=t1t[:])

            def conv_pass(table, epilogue):
                """Aggregate per dst tile from `table`; epilogue(t, agg_psum)."""
                tlo = table[0:HALF, :]
                thi = table[HALF:NPAD, :]
                for g in range(NG):
                    blo = gpool.tile([P, GS * ML * TW], bf16, tag="glo")
                    nc.gpsimd.dma_gather(
                        blo[:].rearrange("p (c d) -> p c d", d=TW),
                        tlo, idxlo[:, g * CLO:(g + 1) * CLO],
                        GS * ML * P, GS * ML * P, TW, single_packet=False,
                        queue_num=(2 * g) % 4)
                    bhi = gpool.tile([P, GS * MH * TW], bf16, tag="ghi")
                    nc.gpsimd.dma_gather(
                        bhi[:].rearrange("p (c d) -> p c d", d=TW),
                        thi, idxhi[:, g * CHI:(g + 1) * CHI],
                        GS * MH * P, GS * MH * P, TW, single_packet=False,
                        queue_num=(2 * g + 1) % 4)
                    blo3 = blo[:].rearrange("p (c d) -> p c d", d=TW)[:, :, 0:HID]
                    wlo_ap = (wvlos[:, g * GS * ML:(g + 1) * GS * ML]
                              [:, :, None].to_broadcast([P, GS * ML, HID]))
                    nc.vector.tensor_tensor(out=blo3, in0=wlo_ap, in1=blo3,
                                            op=Alu.mult)
                    bhi3 = bhi[:].rearrange("p (c d) -> p c d", d=TW)[:, :, 0:HID]
                    whi_ap = (wvhis[:, g * GS * MH:(g + 1) * GS * MH]
                              [:, :, None].to_broadcast([P, GS * MH, HID]))
                    nc.vector.tensor_tensor(out=bhi3, in0=whi_ap, in1=bhi3,
                                            op=Alu.mult)
                    for ti in range(GS):
                        t = g * GS + ti
                        agg = psA.tile([P, HID], f32, tag="agg")
                        for b in range(NB):
                            w4 = min(SB, M2 - b * SB)
                            S4 = spool.tile([P, SB * P], bf16, tag="S")
                            dl_ap = (dls[:, t * M2 + b * SB:
                                         t * M2 + b * SB + w4]
                                     [:, :, None].to_broadcast([P, w4, P]))
                            nc.vector.tensor_tensor(
                                out=S4[:, :w4 * P].rearrange(
                                    "p (c j) -> p c j", j=P),
                                in0=dl_ap,
                                in1=iotaf[:, :w4 * P].rearrange(
                                    "p (c j) -> p c j", j=P),
                                op=Alu.is_equal)
                            for mm_ in range(w4):
                                m = b * SB + mm_
                                if m < ML:
                                    rhs = blo[:, (ti * ML + m) * TW:
                                              (ti * ML + m) * TW + HID]
                                else:
                                    mh = m - ML
                                    rhs = bhi[:, (ti * MH + mh) * TW:
                                              (ti * MH + mh) * TW + HID]
                                nc.tensor.matmul(
                                    agg[:], lhsT=S4[:, mm_ * P:(mm_ + 1) * P],
                                    rhs=rhs, start=(m == 0),
                                    stop=(m == M2 - 1))
                        epilogue(t, agg)

            # ---- phase C: conv1 aggregation + BN stats ----
            st_sum = psS.tile([HID, 1], f32, tag="ssum")
            st_sq = psS.tile([HID, 1], f32, tag="ssq")

            def epi1(t, agg):
                o1 = opool.tile([P, HID], f32, tag="o64")
                tmp = opool.tile([P, HID], f32, tag="o64")
                nc.vector.tensor_scalar(
                    out=tmp[:], in0=agg[:], scalar1=dinvs[:, t:t + 1],
                    scalar2=None, op0=Alu.mult)
                nc.vector.tensor_tensor(out=o1[:], in0=tmp[:], in1=b1bc[:],
                                        op=Alu.add)
                sq = opool.tile([P, HID], f32, tag="o64")
                nc.vector.tensor_tensor(out=sq[:], in0=o1[:], in1=o1[:],
                                        op=Alu.mult)
                mcol = masks[:, 0:1] if t < TPC - 1 else masks[:, 1:2]
                nc.tensor.matmul(st_sum[:], lhsT=o1[:], rhs=mcol,
                                 start=(t == 0), stop=(t == TPC - 1))
                nc.tensor.matmul(st_sq[:], lhsT=sq[:], rhs=mcol,
                                 start=(t == 0), stop=(t == TPC - 1))
                nr = min(P, SHARD - t * P)
                nc.sync.dma_start(out=o1sh[t * P:t * P + nr, :],
                                  in_=o1[:nr, :])

            if ABL_CONV1:
                conv_pass(t1full, epi1)
            else:
                z64 = opool.tile([P, HID], f32, tag="o64")
                nc.gpsimd.memset(z64[:], 0.0)
                for t in range(TPC):
                    mcol = masks[:, 0:1] if t < TPC - 1 else masks[:, 1:2]
                    nc.tensor.matmul(st_sum[:], lhsT=z64[:], rhs=mcol,
                                     start=(t == 0), stop=(t == TPC - 1))
                    nc.tensor.matmul(st_sq[:], lhsT=z64[:], rhs=mcol,
                                     start=(t == 0), stop=(t == TPC - 1))
                    nr = min(P, SHARD - t * P)
                    nc.sync.dma_start(out=o1sh[t * P:t * P + nr, :],
                                      in_=z64[:nr, :])

            st2 = opool.tile([HID, 2], f32, tag="small2")
            nc.vector.tensor_copy(out=st2[:, 0:1], in_=st_sum[:])
            nc.vector.tensor_copy(out=st2[:, 1:2], in_=st_sq[:])
            st2T_p = psB.tile([2, HID], f32, tag="ps")
            nc.tensor.transpose(st2T_p[:], st2[:], ident[:HID, :HID])
            st2T = opool.tile([2, HID], f32, tag="small2T")
            nc.vector.tensor_copy(out=st2T[:], in_=st2T_p[:])
            nc.sync.dma_start(out=o1sh[SHARD:SH2, :], in_=st2T[:])

            if ABL_COLL:
                nc.gpsimd.collective_compute(
                    "AllGather", mybir.AluOpType.bypass, replica_groups=rg,
                    ins=[o1sh.opt()], outs=[o1full[0:NCORES * SH2, :].opt()])
            else:
                for c_ in range(NCORES):
                    nc.sync.dma_start(out=o1full[c_ * SH2:(c_ + 1) * SH2, :],
                                      in_=o1sh[:])

            # ---- global BN stats from the 8 stat-row pairs ----
            o1v = o1full[0:NCORES * SH2, :].rearrange("(c r) d -> c r d",
                                                      c=NCORES)
            sumrows = opool.tile([NCORES, HID], f32, tag="srows")
            nc.sync.dma_start(out=sumrows[:], in_=o1v[:, SHARD, :])
            sqrows = opool.tile([NCORES, HID], f32, tag="srows")
            nc.sync.dma_start(out=sqrows[:], in_=o1v[:, SHARD + 1, :])
            gsum_p = psB.tile([HID, 1], f32, tag="ps")
            nc.tensor.matmul(gsum_p[:], lhsT=sumrows[:],
                             rhs=masks[0:NCORES, 0:1], start=True, stop=True)
            gsq_p = psB.tile([HID, 1], f32, tag="ps")
            nc.tensor.matmul(gsq_p[:], lhsT=sqrows[:],
                             rhs=masks[0:NCORES, 0:1], start=True, stop=True)

            mean = cpool.tile([HID, 1], f32)
            nc.vector.tensor_scalar(out=mean[:], in0=gsum_p[:],
                                    scalar1=1.0 / N, scalar2=None,
                                    op0=Alu.mult)
            var = cpool.tile([HID, 1], f32)
            nc.vector.tensor_scalar(out=var[:], in0=gsq_p[:], scalar1=1.0 / N,
                                    scalar2=None, op0=Alu.mult)
            m2t = opool.tile([HID, 1], f32, tag="small")
            nc.vector.tensor_tensor(out=m2t[:], in0=mean[:], in1=mean[:],
                                    op=Alu.mult)
            nc.vector.tensor_tensor(out=var[:], in0=var[:], in1=m2t[:],
                                    op=Alu.subtract)
            nc.vector.tensor_scalar(out=var[:], in0=var[:], scalar1=BN_EPS,
                                    scalar2=None, op0=Alu.add)
            sd = cpool.tile([HID, 1], f32)
            nc.scalar.activation(out=sd[:], in_=var[:], func=Act.Sqrt)
            rstd = cpool.tile([HID, 1], f32)
            nc.vector.reciprocal(out=rstd[:], in_=sd[:])
            scaleB = cpool.tile([HID, 1], f32)
            nc.vector.tensor_tensor(out=scaleB[:], in0=gc[:], in1=rstd[:],
                                    op=Alu.mult)
            shiftB = cpool.tile([HID, 1], f32)
            nc.vector.tensor_tensor(out=shiftB[:], in0=mean[:], in1=scaleB[:],
                                    op=Alu.mult)
            nc.vector.tensor_tensor(out=shiftB[:], in0=bec[:], in1=shiftB[:],
                                    op=Alu.subtract)
            sBbc_p = psB.tile([P, HID], f32, tag="ps")
            nc.tensor.transpose(sBbc_p[:], scaleB[:].to_broadcast([HID, P]),
                                ident[:HID, :HID])
            sBbc = cpool.tile([P, HID], f32)
            nc.vector.tensor_copy(out=sBbc[:], in_=sBbc_p[:])
            shBbc_p = psB.tile([P, HID], f32, tag="ps")
            nc.tensor.transpose(shBbc_p[:], shiftB[:].to_broadcast([HID, P]),
                                ident[:HID, :HID])
            shBbc = cpool.tile([P, HID], f32)
            nc.vector.tensor_copy(out=shBbc[:], in_=shBbc_p[:])

            # ---- phase C2: t2full = dinv * relu(BN(o1full)), 7 tiles/op ----
            GB = 7
            for c in range(NCORES if ABL_C2 else 1):
                for bi in range(TPC // GB):
                    ct0 = c * TPC + bi * GB
                    src_row = c * SH2 + bi * GB * P
                    dst_row = c * SHARD + bi * GB * P
                    full_rows = (SHARD - bi * GB * P if bi == TPC // GB - 1
                                 else GB * P)
                    o1t = xpool.tile([P, GB * HID], f32, tag="o1t")
                    nc.sync.dma_start(
                        out=o1t[:].rearrange("p (t d) -> p t d", d=HID),
                        in_=o1full[src_row:src_row + GB * P, :].rearrange(
                            "(t p) d -> p t d", p=P))
                    h1 = opool.tile([P, GB * HID], f32, tag="oC2")
                    sB_ap = sBbc[:, None, :].to_broadcast([P, GB, HID])
                    nc.vector.tensor_tensor(
                        out=h1[:].rearrange("p (t d) -> p t d", d=HID),
                        in0=sB_ap,
                        in1=o1t[:].rearrange("p (t d) -> p t d", d=HID),
                        op=Alu.mult)
                    shB_ap = shBbc[:, None, :].to_broadcast([P, GB, HID])
                    nc.vector.tensor_tensor(
                        out=h1[:].rearrange("p (t d) -> p t d", d=HID),
                        in0=shB_ap,
                        in1=h1[:].rearrange("p (t d) -> p t d", d=HID),
                        op=Alu.add)
                    nc.vector.tensor_scalar(
                        out=h1[:], in0=h1[:], scalar1=0.0, scalar2=None,
                        op0=Alu.max)
                    dv_ap = (dinvcb[:, ct0:ct0 + GB]
                             [:, :, None].to_broadcast([P, GB, HID]))
                    t2t = opool.tile([P, GB * HID], bf16, tag="t2t")
                    nc.vector.tensor_tensor(
                        out=t2t[:].rearrange("p (t d) -> p t d", d=HID),
                        in0=dv_ap,
                        in1=h1[:].rearrange("p (t d) -> p t d", d=HID),
                        op=Alu.mult)
                    nfull = full_rows // P
                    if nfull:
                        nc.sync.dma_start(
                            out=t2full[dst_row:dst_row + nfull * P, 0:HID]
                            .rearrange("(t p) d -> p t d", p=P),
                            in_=t2t[:].rearrange("p (t d) -> p t d", d=HID)
                            [:, 0:nfull, :])
                    rem = full_rows - nfull * P
                    if rem:
                        nc.sync.dma_start(
                            out=t2full[dst_row + nfull * P:
                                       dst_row + nfull * P + rem, 0:HID],
                            in_=t2t[:rem, nfull * HID:(nfull + 1) * HID])

            # ---- phase D: conv2 aggregation + W2 + bias ----
            def epi2(t, agg):
                aggs = opool.tile([P, HID], f32, tag="o64")
                nc.vector.tensor_copy(out=aggs[:], in_=agg[:])
                aggT_p = psB.tile([HID, P], f32, tag="ps")
                nc.tensor.transpose(aggT_p[:], aggs[:], ident[:])
                aggT = opool.tile([HID, P], f32, tag="aggT")
                nc.vector.tensor_copy(out=aggT[:], in_=aggT_p[:])
                o2_p = psB.tile([P, OUT_CH], f32, tag="ps")
                nc.tensor.matmul(o2_p[:], lhsT=aggT[:], rhs=w2s[:],
                                 start=True, stop=True)
                o2 = opool.tile([P, OUT_CH], f32, tag="o128")
                nc.vector.tensor_scalar(
                    out=o2[:], in0=o2_p[:], scalar1=dinvs[:, t:t + 1],
                    scalar2=None, op0=Alu.mult)
                nc.vector.tensor_tensor(out=o2[:], in0=o2[:], in1=b2bc[:],
                                        op=Alu.add)
                nc.sync.dma_start(out=y_out[t * P:(t + 1) * P, :], in_=o2[:])

            if ABL_CONV2:
                conv_pass(t2full, epi2)
            else:
                zo = opool.tile([P, OUT_CH], f32, tag="o128")
                nc.gpsimd.memset(zo[:], 0.0)
                for t in range(TPC):
                    nc.sync.dma_start(out=y_out[t * P:(t + 1) * P, :],
                                      in_=zo[:])

    nc.compile()
    return nc


def _make_in_maps(x, W1, b1, bn_gamma, bn_beta, W2, b2, prep):
    import ml_dtypes
    (idxlo_h, idxhi_h, dl_h, wvlo_h, wvhi_h, deg_h, degall_cb, degall_pl,
     ML, MH) = prep
    mask = np.zeros((P, 2), dtype=np.float32)
    mask[:, 0] = 1.0
    mask[:NLAST, 1] = 1.0

    xp = np.zeros((NPAD, IN_CH), dtype=ml_dtypes.bfloat16)
    xp[:N] = x.astype(ml_dtypes.bfloat16)
    w1b = W1.astype(ml_dtypes.bfloat16)

    in_maps = []
    for c in range(NCORES):
        in_maps.append({
            "x_in": xp,
            "idxlo_in": idxlo_h[c],
            "idxhi_in": idxhi_h[c],
            "dl_in": dl_h[c],
            "wvlo_in": wvlo_h[c],
            "wvhi_in": wvhi_h[c],
            "deg_in": deg_h[c],
            "degcb_in": degall_cb,
            "degpl_in": degall_pl,
            "mask_in": mask,
            "w1_in": w1b,
            "w2_in": W2,
            "b1_in": b1.reshape(HID, 1),
            "b2_in": b2.reshape(OUT_CH, 1),
            "g_in": bn_gamma.reshape(HID, 1),
            "be_in": bn_beta.reshape(HID, 1),
        })
    return in_maps


def kernel(x, edge_src, edge_dst, edge_weight, W1, b1, bn_gamma, bn_beta,
           W2, b2):
    global LAST_RESULTS
    from concourse.bass_utils import run_bass_kernel_spmd

    x = np.asarray(x, dtype=np.float32)
    W1 = np.asarray(W1, dtype=np.float32)
    W2 = np.asarray(W2, dtype=np.float32)
    b1 = np.asarray(b1, dtype=np.float32)
    b2 = np.asarray(b2, dtype=np.float32)
    bn_gamma = np.asarray(bn_gamma, dtype=np.float32)
    bn_beta = np.asarray(bn_beta, dtype=np.float32)

    prep = _host_prep(np.asarray(edge_src), np.asarray(edge_dst),
                      np.asarray(edge_weight, dtype=np.float32))
    ML, MH = prep[-2], prep[-1]

    key = (ML, MH, ABL_PHASEB, ABL_CONV1, ABL_C2, ABL_CONV2, ABL_COLL)
    if key not in _PROGRAM_CACHE:
        _PROGRAM_CACHE[key] = _build_program(ML, MH)
    nc = _PROGRAM_CACHE[key]

    in_maps = _make_in_maps(x, W1, b1, bn_gamma, bn_beta, W2, b2, prep)

    res = run_bass_kernel_spmd(nc, in_maps, core_ids=list(range(NCORES)),
                               trace=bool(int(os.environ.get("GCN_TRACE", "0"))))
    LAST_RESULTS = res

    out = np.empty((N, OUT_CH), dtype=np.float32)
    for c in range(NCORES):
        out[c * SHARD:(c + 1) * SHARD] = res.results[c]["y_out"][:SHARD]
    return out


# revision 13
# speedup vs baseline: 3.8835x; 3.8835x over previous
"""GCN embedding kernel for 8 Trainium2 NeuronCores.

Strategy (edge-cut node partitioning, pull model):
  - Nodes are sharded contiguously across 8 cores (6250 each). Each core owns
    the edges whose *destination* lies in its shard.
  - Per core, edges are grouped by 128-node dst tile (49 tiles). Because
    dma_gather indices are int16, each tile's edges are split into "low"
    (src < 32768) and "high" (src >= 32768) segments; each segment is padded
    to a uniform ML / MH chunks of 128 edges so every core runs the identical
    SPMD program.
  - Tables are stored bf16 with 128 columns (64 real + 64 pad) so each row is
    the 256B minimum dma_gather element. Gathers are batched over groups of 7
    dst tiles (one call per table half per group) round-robined over 4 SWDGE
    queues; gathered edge i lands at partition i%128, free block i//128. Each
    gather buffer is scaled by the per-edge weight in one broadcast
    tensor_tensor op.
  - Aggregation: one-hot bf16 selection matrices S[e, j] = (dstlocal_e == j)
    are built four 128-edge chunks at a time with a single broadcast is_equal
    against an iota tile; the scatter-add becomes bf16 PE matmuls
    S^T @ scaled_rows accumulated in fp32 PSUM.
  - Tables are pre-scaled by dinv so the per-edge norm reduces to w_e, with
    dinv[dst] applied on the aggregated output tile (symmetric-norm
    factorization: out[d] = dinv[d] * sum_e w_e * dinv[src_e] * h[src_e]).
  - conv2 aggregates the 64-dim h1 first and applies W2 afterwards
    (segment-sum is linear), halving conv2 gather traffic.
  - table1 = dinv*(x@W1) is computed fully on every core (replicated) from a
    bf16 copy of x using DMA-transpose loads - cheaper than an AllGather.
  - Halo exchange: a single AllGather of the conv1 output shard with the
    per-shard BN statistics appended as two extra rows (no separate
    AllReduce); each core then computes BN+relu+dinv for the full graph
    locally (7 tiles per vector op) to form the conv2 table.
"""

import os

import numpy as np

N = 50000
IN_CH = 128
HID = 64
OUT_CH = 128
NCORES = 8
SHARD = N // NCORES  # 6250
P = 128
TPC = (SHARD + P - 1) // P  # 49 tiles per core
NLAST = SHARD - (TPC - 1) * P  # 106 rows in last tile
BN_EPS = 1e-5
HALF = 32768  # int16 index limit for dma_gather
GS = 7  # dst tiles per gather group
NG = TPC // GS  # 7 groups
SH2 = SHARD + 2  # shard rows + [sum; sumsq] stat rows
SB = 4  # chunks per S-build op
NPAD = 50048  # N padded to tile multiple
GT = NPAD // P  # 391 plain global tiles
TW = 128  # table row width (bf16): 64 data + 64 pad = 256B

LAST_RESULTS = None

# ablation knobs for benchmarking (affect program build)
ABL_PHASEB = True
ABL_CONV1 = True
ABL_C2 = True
ABL_CONV2 = True
ABL_COLL = True

_PROGRAM_CACHE = {}


def _wrap16(flat):
    """dma_gather index layout: linear i -> (partition i%16, col i//16),
    replicated to all 128 partitions (8 Q7 cores x 16 partitions)."""
    arr = flat.reshape(-1, 16).T  # [16, n/16]
    return np.ascontiguousarray(np.tile(arr, (8, 1)))  # [128, n/16]


def _host_prep(edge_src, edge_dst, edge_weight):
    """Partition/sort/pad the edge list. Returns per-core arrays and ML/MH."""
    src = np.concatenate([edge_src.astype(np.int64), np.arange(N, dtype=np.int64)])
    dst = np.concatenate([edge_dst.astype(np.int64), np.arange(N, dtype=np.int64)])
    w = np.concatenate(
        [edge_weight.astype(np.float32), np.ones(N, dtype=np.float32)]
    )

    deg = np.bincount(dst, weights=w.astype(np.float64), minlength=N).astype(
        np.float32
    )

    core = dst // SHARD
    ld = dst - core * SHARD
    t = ld // P
    dl = (ld - t * P).astype(np.float32)
    hi = (src >= HALF).astype(np.int64)

    order = np.lexsort((src, hi, t, core))
    src_s, w_s, dl_s = src[order], w[order], dl[order]
    key2 = ((core[order] * TPC + t[order]) * 2 + hi[order])

    counts = np.bincount(key2, minlength=NCORES * TPC * 2)
    ML = int((counts[0::2].max() + P - 1) // P)
    MH = int((counts[1::2].max() + P - 1) // P)
    M2 = ML + MH

    CM = NCORES * TPC
    seg_starts = np.zeros(NCORES * TPC * 2, dtype=np.int64)
    seg_starts[1:] = np.cumsum(counts)[:-1]
    within = np.arange(len(key2), dtype=np.int64) - seg_starts[key2]
    ct = key2 // 2
    pos = ct * (M2 * P) + np.where(key2 % 2 == 0, 0, ML * P) + within

    SRC = np.zeros(CM * M2 * P, dtype=np.int64)
    DL = np.zeros(CM * M2 * P, dtype=np.float32)
    WV = np.zeros(CM * M2 * P, dtype=np.float32)
    SRC[pos] = src_s
    DL[pos] = dl_s
    WV[pos] = w_s

    SRC = SRC.reshape(NCORES, TPC, M2, P)
    DL = DL.reshape(NCORES, TPC, M2, P)
    WV = WV.reshape(NCORES, TPC, M2, P)

    SRC[:, :, ML:, :] = np.maximum(SRC[:, :, ML:, :] - HALF, 0)

    idxlo_h, idxhi_h, dl_h, wvlo_h, wvhi_h = [], [], [], [], []
    for c in range(NCORES):
        lo = SRC[c, :, :ML, :].reshape(NG, GS * ML * P)
        hi_ = SRC[c, :, ML:, :].reshape(NG, GS * MH * P)
        idxlo_h.append(
            np.concatenate([_wrap16(lo[g].astype(np.int16)) for g in range(NG)],
                           axis=1))
        idxhi_h.append(
            np.concatenate([_wrap16(hi_[g].astype(np.int16)) for g in range(NG)],
                           axis=1))
        dl_h.append(np.ascontiguousarray(DL[c].reshape(TPC * M2, P).T))
        import ml_dtypes
        wvlo_h.append(np.ascontiguousarray(
            WV[c, :, :ML, :].reshape(TPC * ML, P).T).astype(ml_dtypes.bfloat16))
        wvhi_h.append(np.ascontiguousarray(
            WV[c, :, ML:, :].reshape(TPC * MH, P).T).astype(ml_dtypes.bfloat16))

    deg_h = []
    for c in range(NCORES):
        d = deg[c * SHARD:(c + 1) * SHARD]
        dp = np.ones(TPC * P, dtype=np.float32)
        dp[:SHARD] = d
        deg_h.append(np.ascontiguousarray(dp.reshape(TPC, P).T))

    # full-graph deg in core-block tile layout [128, c*TPC+t] (for phase C2)
    dall = np.ones(NCORES * TPC * P, dtype=np.float32)
    for c in range(NCORES):
        dall[c * TPC * P: c * TPC * P + SHARD] = deg[c * SHARD:(c + 1) * SHARD]
    degall_cb = np.ascontiguousarray(dall.reshape(NCORES * TPC, P).T)

    # full-graph deg in plain global tile layout [128, GT] (for phase B)
    dpl = np.ones(NPAD, dtype=np.float32)
    dpl[:N] = deg
    degall_pl = np.ascontiguousarray(dpl.reshape(GT, P).T)

    return (idxlo_h, idxhi_h, dl_h, wvlo_h, wvhi_h, deg_h, degall_cb,
            degall_pl, ML, MH)


def _build_program(ML, MH):
    import concourse.bacc as bacc
    import concourse.mybir as mybir
    import concourse.tile as tile
    from concourse.masks import make_identity

    f32 = mybir.dt.float32
    bf16 = mybir.dt.bfloat16
    i16 = mybir.dt.int16
    i32 = mybir.dt.int32
    Alu = mybir.AluOpType
    Act = mybir.ActivationFunctionType

    M2 = ML + MH
    CLO = GS * ML * P // 16
    CHI = GS * MH * P // 16
    CT = NCORES * TPC  # 392 core-block tiles

    nc = bacc.Bacc("TRN2", target_bir_lowering=False, debug=False,
                   num_devices=NCORES, num_swdge_queues=4)

    x_in = nc.dram_tensor("x_in", [NPAD, IN_CH], bf16, kind="ExternalInput")
    idxlo_in = nc.dram_tensor("idxlo_in", [P, NG * CLO], i16,
                              kind="ExternalInput")
    idxhi_in = nc.dram_tensor("idxhi_in", [P, NG * CHI], i16,
                              kind="ExternalInput")
    dl_in = nc.dram_tensor("dl_in", [P, TPC * M2], f32, kind="ExternalInput")
    wvlo_in = nc.dram_tensor("wvlo_in", [P, TPC * ML], bf16,
                             kind="ExternalInput")
    wvhi_in = nc.dram_tensor("wvhi_in", [P, TPC * MH], bf16,
                             kind="ExternalInput")
    deg_in = nc.dram_tensor("deg_in", [P, TPC], f32, kind="ExternalInput")
    degcb_in = nc.dram_tensor("degcb_in", [P, CT], f32, kind="ExternalInput")
    degpl_in = nc.dram_tensor("degpl_in", [P, GT], f32, kind="ExternalInput")
    mask_in = nc.dram_tensor("mask_in", [P, 2], f32, kind="ExternalInput")
    w1_in = nc.dram_tensor("w1_in", [IN_CH, HID], bf16, kind="ExternalInput")
    w2_in = nc.dram_tensor("w2_in", [HID, OUT_CH], f32, kind="ExternalInput")
    b1_in = nc.dram_tensor("b1_in", [HID, 1], f32, kind="ExternalInput")
    b2_in = nc.dram_tensor("b2_in", [OUT_CH, 1], f32, kind="ExternalInput")
    g_in = nc.dram_tensor("g_in", [HID, 1], f32, kind="ExternalInput")
    be_in = nc.dram_tensor("be_in", [HID, 1], f32, kind="ExternalInput")

    y_out = nc.dram_tensor("y_out", [TPC * P, OUT_CH], f32,
                           kind="ExternalOutput")

    rg = [list(range(NCORES))]
    NB = (M2 + SB - 1) // SB

    with tile.TileContext(nc) as tc:
        with (
            tc.tile_pool(name="const", bufs=1) as cpool,
            tc.tile_pool(name="dram", bufs=1, space="DRAM") as dpool,
            tc.tile_pool(name="xb", bufs=4) as xpool,
            tc.tile_pool(name="gb", bufs=2) as gpool,
            tc.tile_pool(name="sb", bufs=6) as spool,
            tc.tile_pool(name="ob", bufs=6) as opool,
            tc.tile_pool(name="psA", bufs=3, space="PSUM") as psA,
            tc.tile_pool(name="psB", bufs=3, space="PSUM") as psB,
            tc.tile_pool(name="psS", bufs=1, space="PSUM") as psS,
        ):
            # ---- constants / persistent state ----
            w1s = cpool.tile([IN_CH, HID], bf16)
            nc.sync.dma_start(out=w1s[:], in_=w1_in[:])
            w2s = cpool.tile([HID, OUT_CH], f32)
            nc.sync.dma_start(out=w2s[:], in_=w2_in[:])
            b1c = cpool.tile([HID, 1], f32)
            nc.sync.dma_start(out=b1c[:], in_=b1_in[:])
            b2c = cpool.tile([OUT_CH, 1], f32)
            nc.sync.dma_start(out=b2c[:], in_=b2_in[:])
            gc = cpool.tile([HID, 1], f32)
            nc.sync.dma_start(out=gc[:], in_=g_in[:])
            bec = cpool.tile([HID, 1], f32)
            nc.sync.dma_start(out=bec[:], in_=be_in[:])
            idxlo = cpool.tile([P, NG * CLO], i16)
            nc.sync.dma_start(out=idxlo[:], in_=idxlo_in[:])
            idxhi = cpool.tile([P, NG * CHI], i16)
            nc.sync.dma_start(out=idxhi[:], in_=idxhi_in[:])
            dls = cpool.tile([P, TPC * M2], f32)
            nc.sync.dma_start(out=dls[:], in_=dl_in[:])
            wvlos = cpool.tile([P, TPC * ML], bf16)
            nc.sync.dma_start(out=wvlos[:], in_=wvlo_in[:])
            wvhis = cpool.tile([P, TPC * MH], bf16)
            nc.sync.dma_start(out=wvhis[:], in_=wvhi_in[:])
            masks = cpool.tile([P, 2], f32)
            nc.sync.dma_start(out=masks[:], in_=mask_in[:])

            degs = cpool.tile([P, TPC], f32)
            nc.sync.dma_start(out=degs[:], in_=deg_in[:])
            dinvs = cpool.tile([P, TPC], f32)
            nc.scalar.activation(out=dinvs[:], in_=degs[:], func=Act.Sqrt)
            nc.vector.reciprocal(out=dinvs[:], in_=dinvs[:])
            degcbs = cpool.tile([P, CT], f32)
            nc.sync.dma_start(out=degcbs[:], in_=degcb_in[:])
            dinvcb = cpool.tile([P, CT], f32)
            nc.scalar.activation(out=dinvcb[:], in_=degcbs[:], func=Act.Sqrt)
            nc.vector.reciprocal(out=dinvcb[:], in_=dinvcb[:])
            degpls = cpool.tile([P, GT], f32)
            nc.sync.dma_start(out=degpls[:], in_=degpl_in[:])
            dinvpl = cpool.tile([P, GT], f32)
            nc.scalar.activation(out=dinvpl[:], in_=degpls[:], func=Act.Sqrt)
            nc.vector.reciprocal(out=dinvpl[:], in_=dinvpl[:])

            ident = cpool.tile([P, P], f32)
            make_identity(nc, ident[:])
            iota_i = cpool.tile([P, SB * P], i32)
            nc.gpsimd.iota(iota_i[:], [[0, SB], [1, P]], channel_multiplier=0)
            iotaf = cpool.tile([P, SB * P], f32)
            nc.vector.tensor_copy(out=iotaf[:], in_=iota_i[:])

            b1bc_p = psB.tile([P, HID], f32, tag="ps")
            nc.tensor.transpose(b1bc_p[:], b1c[:].to_broadcast([HID, P]),
                                ident[:HID, :HID])
            b1bc = cpool.tile([P, HID], f32)
            nc.vector.tensor_copy(out=b1bc[:], in_=b1bc_p[:])
            b2bc_p = psB.tile([P, OUT_CH], f32, tag="ps")
            nc.tensor.transpose(b2bc_p[:], b2c[:].to_broadcast([OUT_CH, P]),
                                ident[:])
            b2bc = cpool.tile([P, OUT_CH], f32)
            nc.vector.tensor_copy(out=b2bc[:], in_=b2bc_p[:])

            t1full = dpool.tile([NPAD, TW], bf16)
            o1sh = dpool.tile([SH2, HID], f32)
            o1full = dpool.tile([NCORES * SH2 + 32, HID], f32)
            t2full = dpool.tile([NPAD, TW], bf16)

            # ---- phase B (replicated): t1full = dinv * (x @ W1), bf16 ----
            for g in range(GT if ABL_PHASEB else 8):
                xT = xpool.tile([P, P], bf16, tag="xT")
                nc.sync.dma_start(out=xT[:], in_=x_in[g * P:(g + 1) * P, :],
                                  transpose=True)
                h_p = psB.tile([P, HID], f32, tag="ps")
                nc.tensor.matmul(h_p[:], lhsT=xT[:], rhs=w1s[:],
                                 start=True, stop=True)
                t1t = opool.tile([P, HID], bf16, tag="t1t")
                nc.vector.tensor_scalar(
                    out=t1t[:], in0=h_p[:], scalar1=dinvpl[:, g:g + 1],
                    scalar2=None, op0=Alu.mult)
                nc.sync.dma_start(out=t1full[g * P:(g + 1) * P, 0:HID],
                                  in_=t1t[:])

            def conv_pass(table, epilogue):
                """Aggregate per dst tile from `table`; epilogue(t, agg_psum)."""
                tlo = table[0:HALF, :]
                thi = table[HALF:NPAD, :]
                for g in range(NG):
                    blo = gpool.tile([P, GS * ML * TW], bf16, tag="glo")
                    nc.gpsimd.dma_gather(
                        blo[:].rearrange("p (c d) -> p c d", d=TW),
                        tlo, idxlo[:, g * CLO:(g + 1) * CLO],
                        GS * ML * P, GS * ML * P, TW, single_packet=False,
                        queue_num=(2 * g) % 4)
                    bhi = gpool.tile([P, GS * MH * TW], bf16, tag="ghi")
                    nc.gpsimd.dma_gather(
                        bhi[:].rearrange("p (c d) -> p c d", d=TW),
                        thi, idxhi[:, g * CHI:(g + 1) * CHI],
                        GS * MH * P, GS * MH * P, TW, single_packet=False,
                        queue_num=(2 * g + 1) % 4)
                    blo3 = blo[:].rearrange("p (c d) -> p c d", d=TW)[:, :, 0:HID]
                    wlo_ap = (wvlos[:, g * GS * ML:(g + 1) * GS * ML]
                              [:, :, None].to_broadcast([P, GS * ML, HID]))
                    nc.vector.tensor_tensor(out=blo3, in0=wlo_ap, in1=blo3,
                                            op=Alu.mult)
                    bhi3 = bhi[:].rearrange("p (c d) -> p c d", d=TW)[:, :, 0:HID]
                    whi_ap = (wvhis[:, g * GS * MH:(g + 1) * GS * MH]
                              [:, :, None].to_broadcast([P, GS * MH, HID]))
                    nc.vector.tensor_tensor(out=bhi3, in0=whi_ap, in1=bhi3,
                                            op=Alu.mult)
                    for ti in range(GS):
                        t = g * GS + ti
                        agg = psA.tile([P, HID], f32, tag="agg")
                        for b in range(NB):
                            w4 = min(SB, M2 - b * SB)
                            S4 = spool.tile([P, SB * P], bf16, tag="S")
                            dl_ap = (dls[:, t * M2 + b * SB:
                                         t * M2 + b * SB + w4]
                                     [:, :, None].to_broadcast([P, w4, P]))
                            nc.vector.tensor_tensor(
                                out=S4[:, :w4 * P].rearrange(
                                    "p (c j) -> p c j", j=P),
                                in0=dl_ap,
                                in1=iotaf[:, :w4 * P].rearrange(
                                    "p (c j) -> p c j", j=P),
                                op=Alu.is_equal)
                            for mm_ in range(w4):
                                m = b * SB + mm_
                                if m < ML:
                                    rhs = blo[:, (ti * ML + m) * TW:
                                              (ti * ML + m) * TW + HID]
                                else:
                                    mh = m - ML
                                    rhs = bhi[:, (ti * MH + mh) * TW:
                                              (ti * MH + mh) * TW + HID]
                                nc.tensor.matmul(
                                    agg[:], lhsT=S4[:, mm_ * P:(mm_ + 1) * P],
                                    rhs=rhs, start=(m == 0),
                                    stop=(m == M2 - 1))
                        epilogue(t, agg)

            # ---- phase C: conv1 aggregation + BN stats ----
            st_sum = psS.tile([HID, 1], f32, tag="ssum")
            st_sq = psS.tile([HID, 1], f32, tag="ssq")

            def epi1(t, agg):
                o1 = opool.tile([P, HID], f32, tag="o64")
                tmp = opool.tile([P, HID], f32, tag="o64")
                nc.vector.tensor_scalar(
                    out=tmp[:], in0=agg[:], scalar1=dinvs[:, t:t + 1],
                    scalar2=None, op0=Alu.mult)
                nc.vector.tensor_tensor(out=o1[:], in0=tmp[:], in1=b1bc[:],
                                        op=Alu.add)
                sq = opool.tile([P, HID], f32, tag="o64")
                nc.vector.tensor_tensor(out=sq[:], in0=o1[:], in1=o1[:],
                                        op=Alu.mult)
                mcol = masks[:, 0:1] if t < TPC - 1 else masks[:, 1:2]
                nc.tensor.matmul(st_sum[:], lhsT=o1[:], rhs=mcol,
                                 start=(t == 0), stop=(t == TPC - 1))
                nc.tensor.matmul(st_sq[:], lhsT=sq[:], rhs=mcol,
                                 start=(t == 0), stop=(t == TPC - 1))
                nr = min(P, SHARD - t * P)
                nc.sync.dma_start(out=o1sh[t * P:t * P + nr, :],
                                  in_=o1[:nr, :])

            if ABL_CONV1:
                conv_pass(t1full, epi1)
            else:
                z64 = opool.tile([P, HID], f32, tag="o64")
                nc.gpsimd.memset(z64[:], 0.0)
                for t in range(TPC):
                    mcol = masks[:, 0:1] if t < TPC - 1 else masks[:, 1:2]
                    nc.tensor.matmul(st_sum[:], lhsT=z64[:], rhs=mcol,
                                     start=(t == 0), stop=(t == TPC - 1))
                    nc.tensor.matmul(st_sq[:], lhsT=z64[:], rhs=mcol,
                                     start=(t == 0), stop=(t == TPC - 1))
                    nr = min(P, SHARD - t * P)
                    nc.sync.dma_start(out=o1sh[t * P:t * P + nr, :],
                                      in_=z64[:nr, :])

            st2 = opool.tile([HID, 2], f32, tag="small2")
            nc.vector.tensor_copy(out=st2[:, 0:1], in_=st_sum[:])
            nc.vector.tensor_copy(out=st2[:, 1:2], in_=st_sq[:])
            st2T_p = psB.tile([2, HID], f32, tag="ps")
            nc.tensor.transpose(st2T_p[:], st2[:], ident[:HID, :HID])
            st2T = opool.tile([2, HID], f32, tag="small2T")
            nc.vector.tensor_copy(out=st2T[:], in_=st2T_p[:])
            nc.sync.dma_start(out=o1sh[SHARD:SH2, :], in_=st2T[:])

            if ABL_COLL:
                nc.gpsimd.collective_compute(
                    "AllGather", mybir.AluOpType.bypass, replica_groups=rg,
                    ins=[o1sh.opt()], outs=[o1full[0:NCORES * SH2, :].opt()])
            else:
                for c_ in range(NCORES):
                    nc.sync.dma_start(out=o1full[c_ * SH2:(c_ + 1) * SH2, :],
                                      in_=o1sh[:])

            # ---- global BN stats from the 8 stat-row pairs ----
            o1v = o1full[0:NCORES * SH2, :].rearrange("(c r) d -> c r d",
                                                      c=NCORES)
            sumrows = opool.tile([NCORES, HID], f32, tag="srows")
            nc.sync.dma_start(out=sumrows[:], in_=o1v[:, SHARD, :])
            sqrows = opool.tile([NCORES, HID], f32, tag="srows")
            nc.sync.dma_start(out=sqrows[:], in_=o1v[:, SHARD + 1, :])
            gsum_p = psB.tile([HID, 1], f32, tag="ps")
            nc.tensor.matmul(gsum_p[:], lhsT=sumrows[:],
                             rhs=masks[0:NCORES, 0:1], start=True, stop=True)
            gsq_p = psB.tile([HID, 1], f32, tag="ps")
            nc.tensor.matmul(gsq_p[:], lhsT=sqrows[:],
                             rhs=masks[0:NCORES, 0:1], start=True, stop=True)

            mean = cpool.tile([HID, 1], f32)
            nc.vector.tensor_scalar(out=mean[:], in0=gsum_p[:],
                                    scalar1=1.0 / N, scalar2=None,
                                    op0=Alu.mult)
            var = cpool.tile([HID, 1], f32)
            nc.vector.tensor_scalar(out=var[:], in0=gsq_p[:], scalar1=1.0 / N,
                                    scalar2=None, op0=Alu.mult)
            m2t = opool.tile([HID, 1], f32, tag="small")
            nc.vector.tensor_tensor(out=m2t[:], in0=mean[:], in1=mean[:],
                                    op=Alu.mult)
            nc.vector.tensor_tensor(out=var[:], in0=var[:], in1=m2t[:],
                                    op=Alu.subtract)
            nc.vector.tensor_scalar(out=var[:], in0=var[:], scalar1=BN_EPS,
                                    scalar2=None, op0=Alu.add)
            sd = cpool.tile([HID, 1], f32)
            nc.scalar.activation(out=sd[:], in_=var[:], func=Act.Sqrt)
            rstd = cpool.tile([HID, 1], f32)
            nc.vector.reciprocal(out=rstd[:], in_=sd[:])
            scaleB = cpool.tile([HID, 1], f32)
            nc.vector.tensor_tensor(out=scaleB[:], in0=gc[:], in1=rstd[:],
                                    op=Alu.mult)
            shiftB = cpool.tile([HID, 1], f32)
            nc.vector.tensor_tensor(out=shiftB[:], in0=mean[:], in1=scaleB[:],
                                    op=Alu.mult)
            nc.vector.tensor_tensor(out=shiftB[:], in0=bec[:], in1=shiftB[:],
                                    op=Alu.subtract)
            sBbc_p = psB.tile([P, HID], f32, tag="ps")
            nc.tensor.transpose(sBbc_p[:], scaleB[:].to_broadcast([HID, P]),
                                ident[:HID, :HID])
            sBbc = cpool.tile([P, HID], f32)
            nc.vector.tensor_copy(out=sBbc[:], in_=sBbc_p[:])
            shBbc_p = psB.tile([P, HID], f32, tag="ps")
            nc.tensor.transpose(shBbc_p[:], shiftB[:].to_broadcast([HID, P]),
                                ident[:HID, :HID])
            shBbc = cpool.tile([P, HID], f32)
            nc.vector.tensor_copy(out=shBbc[:], in_=shBbc_p[:])

            # ---- phase C2: t2full = dinv * relu(BN(o1full)), 7 tiles/op ----
            GB = 7
            for c in range(NCORES if ABL_C2 else 1):
                for bi in range(TPC // GB):
                    ct0 = c * TPC + bi * GB
                    src_row = c * SH2 + bi * GB * P
                    dst_row = c * SHARD + bi * GB * P
                    full_rows = (SHARD - bi * GB * P if bi == TPC // GB - 1
                                 else GB * P)
                    o1t = xpool.tile([P, GB * HID], f32, tag="o1t")
                    nc.sync.dma_start(
                        out=o1t[:].rearrange("p (t d) -> p t d", d=HID),
                        in_=o1full[src_row:src_row + GB * P, :].rearrange(
                            "(t p) d -> p t d", p=P))
                    h1 = opool.tile([P, GB * HID], f32, tag="oC2")
                    sB_ap = sBbc[:, None, :].to_broadcast([P, GB, HID])
                    nc.vector.tensor_tensor(
                        out=h1[:].rearrange("p (t d) -> p t d", d=HID),
                        in0=sB_ap,
                        in1=o1t[:].rearrange("p (t d) -> p t d", d=HID),
                        op=Alu.mult)
                    shB_ap = shBbc[:, None, :].to_broadcast([P, GB, HID])
                    nc.vector.tensor_tensor(
                        out=h1[:].rearrange("p (t d) -> p t d", d=HID),
                        in0=shB_ap,
                        in1=h1[:].rearrange("p (t d) -> p t d", d=HID),
                        op=Alu.add)
                    nc.vector.tensor_scalar(
                        out=h1[:], in0=h1[:], scalar1=0.0, scalar2=None,
                        op0=Alu.max)
                    dv_ap = (dinvcb[:, ct0:ct0 + GB]
                             [:, :, None].to_broadcast([P, GB, HID]))
                    t2t = opool.tile([P, GB * HID], bf16, tag="t2t")
                    nc.vector.tensor_tensor(
                        out=t2t[:].rearrange("p (t d) -> p t d", d=HID),
                        in0=dv_ap,
                        in1=h1[:].rearrange("p (t d) -> p t d", d=HID),
                        op=Alu.mult)
                    nfull = full_rows // P
                    if nfull:
                        nc.sync.dma_start(
                            out=t2full[dst_row:dst_row + nfull * P, 0:HID]
                            .rearrange("(t p) d -> p t d", p=P),
                            in_=t2t[:].rearrange("p (t d) -> p t d", d=HID)
                            [:, 0:nfull, :])
                    rem = full_rows - nfull * P
                    if rem:
                        nc.sync.dma_start(
                            out=t2full[dst_row + nfull * P:
                                       dst_row + nfull * P + rem, 0:HID],
                            in_=t2t[:rem, nfull * HID:(nfull + 1) * HID])

            # ---- phase D: conv2 aggregation + W2 + bias ----
            def epi2(t, agg):
                aggs = opool.tile([P, HID], f32, tag="o64")
                nc.vector.tensor_copy(out=aggs[:], in_=agg[:])
                aggT_p = psB.tile([HID, P], f32, tag="ps")
                nc.tensor.transpose(aggT_p[:], aggs[:], ident[:])
                aggT = opool.tile([HID, P], f32, tag="aggT")
                nc.vector.tensor_copy(out=aggT[:], in_=aggT_p[:])
                o2_p = psB.tile([P, OUT_CH], f32, tag="ps")
                nc.tensor.matmul(o2_p[:], lhsT=aggT[:], rhs=w2s[:],
                                 start=True, stop=True)
                o2 = opool.tile([P, OUT_CH], f32, tag="o128")
                nc.vector.tensor_scalar(
                    out=o2[:], in0=o2_p[:], scalar1=dinvs[:, t:t + 1],
                    scalar2=None, op0=Alu.mult)
                nc.vector.tensor_tensor(out=o2[:], in0=o2[:], in1=b2bc[:],
                                        op=Alu.add)
                nc.sync.dma_start(out=y_out[t * P:(t + 1) * P, :], in_=o2[:])

            if ABL_CONV2:
                conv_pass(t2full, epi2)
            else:
                zo = opool.tile([P, OUT_CH], f32, tag="o128")
                nc.gpsimd.memset(zo[:], 0.0)
                for t in range(TPC):
                    nc.sync.dma_start(out=y_out[t * P:(t + 1) * P, :],
                                      in_=zo[:])

    nc.compile()
    return nc


def _make_in_maps(x, W1, b1, bn_gamma, bn_beta, W2, b2, prep):
    import ml_dtypes
    (idxlo_h, idxhi_h, dl_h, wvlo_h, wvhi_h, deg_h, degall_cb, degall_pl,
     ML, MH) = prep
    mask = np.zeros((P, 2), dtype=np.float32)
    mask[:, 0] = 1.0
    mask[:NLAST, 1] = 1.0

    xp = np.zeros((NPAD, IN_CH), dtype=ml_dtypes.bfloat16)
    xp[:N] = x.astype(ml_dtypes.bfloat16)
    w1b = W1.astype(ml_dtypes.bfloat16)

    in_maps = []
    for c in range(NCORES):
        in_maps.append({
            "x_in": xp,
            "idxlo_in": idxlo_h[c],
            "idxhi_in": idxhi_h[c],
            "dl_in": dl_h[c],
            "wvlo_in": wvlo_h[c],
            "wvhi_in": wvhi_h[c],
            "deg_in": deg_h[c],
            "degcb_in": degall_cb,
            "degpl_in": degall_pl,
            "mask_in": mask,
            "w1_in": w1b,
            "w2_in": W2,
            "b1_in": b1.reshape(HID, 1),
            "b2_in": b2.reshape(OUT_CH, 1),
            "g_in": bn_gamma.reshape(HID, 1),
            "be_in": bn_beta.reshape(HID, 1),
        })
    return in_maps


def kernel(x, edge_src, edge_dst, edge_weight, W1, b1, bn_gamma, bn_beta,
           W2, b2):
    global LAST_RESULTS
    from concourse.bass_utils import run_bass_kernel_spmd

    x = np.asarray(x, dtype=np.float32)
    W1 = np.asarray(W1, dtype=np.float32)
    W2 = np.asarray(W2, dtype=np.float32)
    b1 = np.asarray(b1, dtype=np.float32)
    b2 = np.asarray(b2, dtype=np.float32)
    bn_gamma = np.asarray(bn_gamma, dtype=np.float32)
    bn_beta = np.asarray(bn_beta, dtype=np.float32)

    prep = _host_prep(np.asarray(edge_src), np.asarray(edge_dst),
                      np.asarray(edge_weight, dtype=np.float32))
    ML, MH = prep[-2], prep[-1]

    key = (ML, MH, ABL_PHASEB, ABL_CONV1, ABL_C2, ABL_CONV2, ABL_COLL)
    if key not in _PROGRAM_CACHE:
        _PROGRAM_CACHE[key] = _build_program(ML, MH)
    nc = _PROGRAM_CACHE[key]

    in_maps = _make_in_maps(x, W1, b1, bn_gamma, bn_beta, W2, b2, prep)

    res = run_bass_kernel_spmd(nc, in_maps, core_ids=list(range(NCORES)),
                               trace=bool(int(os.environ.get("GCN_TRACE", "0"))))
    LAST_RESULTS = res

    out = np.empty((N, OUT_CH), dtype=np.float32)
    for c in range(NCORES):
        out[c * SHARD:(c + 1) * SHARD] = res.results[c]["y_out"][:SHARD]
    return out
